# revision 31
# baseline (speedup 1.0000x reference)
"""Trainium2 Bass kernel for the GRU encoder problem (nn_Encoder).

Computation: x = embedding[source]; gi = x @ w_ih.T + b_ih; then a GRU
recurrence over T=128 steps producing enc_outputs [T, B, H].

Strategy: data-parallel over batch across 8 NeuronCores (B=64 -> 8 rows/core,
embedding + GRU weights replicated). Inside each core everything runs in a
"transposed" layout with gate/hidden dims on SBUF partitions and batch on the
free dim, so the sequential recurrence's per-step matmuls put gates on PSUM
partitions and the elementwise gate math uses all 128 lanes.

v2 changes vs v1:
  - gi kept resident in SBUF as bf16 (no DRAM round trip, no per-step DMA)
  - bf16 embedding gather / transposes / w_ih (FWL; half the DMA volume)
  - gates reordered on host to (r, n, z) so z's chain is the only tail work:
      h' = n + z*(h - n)
  - single bf16 hidden state; bf16 outputs cast to f32 on host
"""
import numpy as np

V, E, H, B, T = 32000, 1024, 1280, 64, 128
BL = 8            # batch rows per core
G3 = 3 * H        # 3840
NJ = G3 // 128    # 30 gate blocks (after host reorder: 0-9 r, 10-19 n, 20-29 z)
NK = H // 128     # 10 hidden blocks
NE = E // 128     # 8 embedding blocks
N_CORES = 8

_CACHE = {}


def _build(nc):
    import concourse.mybir as mybir
    import concourse.tile as tile

    F32 = mybir.dt.float32
    BF16 = mybir.dt.bfloat16
    I16 = mybir.dt.int16
    AF = mybir.ActivationFunctionType
    OP = mybir.AluOpType

    NTOK = T * BL
    NTC = NTOK // 128

    emb_d = nc.dram_tensor("emb", [V, E], BF16, kind="ExternalInput")
    idx_d = nc.dram_tensor("idx", [128, NTOK // 16], I16, kind="ExternalInput")
    wihT_d = nc.dram_tensor("wihT", [E, G3], BF16, kind="ExternalInput")
    whhT_d = nc.dram_tensor("whhT", [H, G3], BF16, kind="ExternalInput")
    bgi_d = nc.dram_tensor("bgi", [128, NJ], F32, kind="ExternalInput")
    bhhn_d = nc.dram_tensor("bhhn", [128, BL * NK], BF16, kind="ExternalInput")
    ident_d = nc.dram_tensor("ident", [128, 128], BF16, kind="ExternalInput")
    outT_d = nc.dram_tensor("outT", [128, T, BL * NK], BF16,
                            kind="ExternalOutput")

    with tile.TileContext(nc) as tc:
        with tc.tile_pool(name="const", bufs=1) as cpool:
            bgi = cpool.tile([128, NJ], F32, tag="bgi")
            nc.sync.dma_start(bgi[:, :], bgi_d.ap())
            bhhn = cpool.tile([128, BL * NK], BF16, tag="bhhn")
            nc.sync.dma_start(bhhn[:, :], bhhn_d.ap())
            ident = cpool.tile([128, 128], BF16, tag="ident")
            nc.sync.dma_start(ident[:, :], ident_d.ap())
            giT = cpool.tile([128, NJ, NTOK], BF16, tag="giT")
            # whh loaded up front: its DMA overlaps the gather + gi GEMM
            whh = cpool.tile([128, NK, G3], BF16, tag="whh")
            for k in range(NK):
                nc.sync.dma_start(
                    whh[:, k, :], whhT_d.ap()[128 * k:128 * (k + 1), :])

            # ---------- phases A-D: gather, transpose, gi GEMM ----------
            with tc.tile_pool(name="gemm", bufs=1) as gp:
                idx_sb = gp.tile([128, NTOK // 16], I16, tag="idx")
                nc.sync.dma_start(idx_sb[:, :], idx_d.ap())
                xT = gp.tile([128, NE, NTOK], BF16, tag="xT")
                with tc.tile_pool(name="xp", bufs=1) as xp:
                    x_sb = xp.tile([128, NTC, E], BF16, tag="x")
                    nc.gpsimd.dma_gather(
                        x_sb[:, :, :], emb_d.ap(), idx_sb[:, :],
                        num_idxs=NTOK, num_idxs_reg=NTOK, elem_size=E)
                    with tc.tile_pool(name="trps", bufs=4, space="PSUM") as tpp:
                        for c in range(NTC):
                            for e in range(NE):
                                tps = tpp.tile([128, 128], BF16, tag="tps")
                                nc.tensor.transpose(
                                    tps[:, :], x_sb[:, c, 128 * e:128 * (e + 1)],
                                    ident[:, :])
                                nc.vector.tensor_copy(
                                    xT[:, e, 128 * c:128 * (c + 1)], tps[:, :])
                NN = NTOK // 512
                NW = 512
                with tc.tile_pool(name="gips", bufs=4, space="PSUM") as gpp:
                    for j in range(NJ):
                        wjs = gp.tile([128, NE, 128], BF16, tag="wjs", bufs=3)
                        nc.sync.dma_start(
                            wjs[:, :, :],
                            wihT_d.ap()[:, 128 * j:128 * (j + 1)]
                            .rearrange("(e p) c -> p e c", p=128))
                        for n in range(NN):
                            gps = gpp.tile([128, NW], F32, tag="gps")
                            for e in range(NE):
                                nc.tensor.matmul(
                                    gps[:, :],
                                    wjs[:, e, :],
                                    xT[:, e, NW * n:NW * (n + 1)],
                                    start=(e == 0), stop=(e == NE - 1))
                            nc.scalar.activation(
                                giT[:, j, NW * n:NW * (n + 1)], gps[:, :],
                                AF.Identity, bias=bgi[:, j:j + 1])

            # ---------- phase E: recurrence (bf16 weights/stream) ----------
            with tc.tile_pool(name="rec", bufs=1) as rp:
                out_sb = rp.tile([128, T + 1, BL * NK], BF16, tag="osb")
                with tc.tile_pool(name="recw", bufs=4) as rw, \
                     tc.tile_pool(name="gr_ps", bufs=2, space="PSUM") as grp, \
                     tc.tile_pool(name="gn_ps", bufs=2, space="PSUM") as gnp, \
                     tc.tile_pool(name="gz_ps", bufs=2, space="PSUM") as gzp:
                    nc.vector.memset(out_sb[:, 0, :], 0.0)
                    for t in range(T):
                        G_r = grp.tile([128, 10 * BL], F32, tag="gr")
                        G_n = gnp.tile([128, 10 * BL], F32, tag="gn")
                        G_z = gzp.tile([128, 10 * BL], F32, tag="gz")
                        for g, G in enumerate((G_r, G_n, G_z)):
                            # seed PSUM with the elementwise-add operand via
                            # one identity-stationary matmul over all 10
                            # blocks (doesn't depend on hc, so it can issue
                            # ahead of the weight matmuls)
                            add_sl = (bhhn[:, :].rearrange(
                                "p (j b) -> p j b", b=BL) if g == 1 else
                                giT[:, 10 * g:10 * (g + 1), BL * t:BL * (t + 1)])
                            nc.tensor.matmul(
                                G[:, :].rearrange("p (j b) -> p j b", b=BL),
                                ident[:, :], add_sl,
                                start=True, stop=False, skip_group_check=True)
                            for j10 in range(10):
                                j = 10 * g + j10
                                out_sl = G[:, BL * j10:BL * (j10 + 1)]
                                for k in range(NK):
                                    nc.tensor.matmul(
                                        out_sl,
                                        whh[:, k, 128 * j:128 * (j + 1)],
                                        out_sb[:, t, BL * k:BL * (k + 1)],
                                        start=False,
                                        stop=(j10 == 9 and k == NK - 1),
                                        skip_group_check=True)
                        # r chain (hidden under n-block matmuls)
                        r_g = rw.tile([128, 10 * BL], F32, tag="rg")
                        nc.scalar.activation(r_g[:, :], G_r[:, :], AF.Sigmoid)
                        # n chain (hidden under z-block matmuls)
                        t2 = rw.tile([128, 10 * BL], F32, tag="t2")
                        nc.vector.tensor_tensor(
                            t2[:, :], G_n[:, :], r_g[:, :], OP.mult)
                        npre = rw.tile([128, 10 * BL], F32, tag="npre")
                        nc.vector.tensor_tensor(
                            npre[:, :].rearrange("p (j b) -> p j b", b=BL),
                            t2[:, :].rearrange("p (j b) -> p j b", b=BL),
                            giT[:, 10:20, BL * t:BL * (t + 1)], OP.add)
                        n_g = rw.tile([128, 10 * BL], F32, tag="ng")
                        nc.scalar.activation(n_g[:, :], npre[:, :], AF.Tanh)
                        d_t = rw.tile([128, 10 * BL], F32, tag="dt")
                        nc.vector.tensor_tensor(
                            d_t[:, :], out_sb[:, t, :], n_g[:, :], OP.subtract)
                        # z chain: the only tail after the last matmul
                        z_g = rw.tile([128, 10 * BL], F32, tag="zg")
                        nc.scalar.activation(z_g[:, :], G_z[:, :], AF.Sigmoid)
                        t3 = rw.tile([128, 10 * BL], F32, tag="t3")
                        nc.vector.tensor_tensor(
                            t3[:, :], z_g[:, :], d_t[:, :], OP.mult)
                        nc.vector.tensor_tensor(
                            out_sb[:, t + 1, :], t3[:, :], n_g[:, :], OP.add)
                        if t % 16 == 15:
                            nc.sync.dma_start(
                                outT_d.ap()[:, t - 15:t + 1, :],
                                out_sb[:, t - 14:t + 2, :])


class _Compiled:
    def __init__(self):
        import jax
        import numpy as _np
        import concourse.bacc as bacc
        import concourse.mybir as mybir
        from jax.sharding import Mesh, PartitionSpec, NamedSharding
        from jax.experimental.shard_map import shard_map
        from concourse.bass2jax import (
            _bass_exec_p, partition_id_tensor, install_neuronx_cc_hook)

        install_neuronx_cc_hook()
        nc = bacc.Bacc("TRN2", target_bir_lowering=False, debug=False,
                       enable_asserts=True, num_devices=1)
        _build(nc)
        nc.compile()
        self.nc = nc
        self.jax = jax

        partition_name = (nc.partition_id_tensor.name
                          if nc.partition_id_tensor else None)
        in_names, out_names, out_avals, zero_outs = [], [], [], []
        for alloc in nc.m.functions[0].allocations:
            if not isinstance(alloc, mybir.MemoryLocationSet):
                continue
            name = alloc.memorylocations[0].name
            if alloc.kind == "ExternalInput":
                if name != partition_name:
                    in_names.append(name)
            elif alloc.kind == "ExternalOutput":
                out_names.append(name)
                shape = tuple(alloc.tensor_shape)
                dt = mybir.dt.np(alloc.dtype)
                out_avals.append(jax.core.ShapedArray(shape, dt))
                zero_outs.append(_np.zeros(shape, dt))
        self.in_params = list(in_names)
        self.out_names = out_names
        self.out_avals = out_avals
        n_params = len(in_names)
        in_names = in_names + out_names
        if partition_name is not None:
            in_names.append(partition_name)

        def _body(*args):
            args = list(args)
            if partition_name is not None:
                args.append(partition_id_tensor())
            outs = _bass_exec_p.bind(
                *args, out_avals=tuple(out_avals), in_names=tuple(in_names),
                out_names=tuple(out_names), lowering_input_output_aliases=(),
                sim_require_finite=True, sim_require_nnan=True, nc=nc)
            return tuple(outs)

        devices = jax.devices()[:N_CORES]
        mesh = Mesh(_np.asarray(devices), ("core",))
        n_in = n_params + len(out_names)
        self.sharded = jax.jit(
            shard_map(_body, mesh=mesh,
                      in_specs=(PartitionSpec("core"),) * n_in,
                      out_specs=(PartitionSpec("core"),) * len(out_names),
                      check_rep=False),
            keep_unused=True)
        self.sh = NamedSharding(mesh, PartitionSpec("core"))
        self.zero_outs = zero_outs

    def put_inputs(self, in_maps):
        import numpy as _np
        jax = self.jax
        concat = [_np.concatenate([_np.ascontiguousarray(in_maps[c][n])
                                   for c in range(N_CORES)], axis=0)
                  for n in self.in_params]
        args = [jax.device_put(a, self.sh) for a in concat]
        zeros = [jax.device_put(
            _np.zeros((N_CORES * z.shape[0], *z.shape[1:]), z.dtype), self.sh)
            for z in self.zero_outs]
        return args + zeros

    def run(self, dev_args):
        out = self.sharded(*dev_args)
        self.jax.block_until_ready(out)
        return out

    def results(self, out):
        import numpy as _np
        res = []
        for c in range(N_CORES):
            d = {}
            for i, name in enumerate(self.out_names):
                a = _np.asarray(out[i])
                d[name] = a.reshape(N_CORES, *self.out_avals[i].shape)[c]
            res.append(d)
        return res


def _get_compiled():
    if "k" not in _CACHE:
        _CACHE["k"] = _Compiled()
    return _CACHE["k"]


def _prep_core_inputs(source_core, embedding, wihT, whhT, bgi, bhhn, ident):
    NTOK = T * BL
    idx_lin = source_core.T.reshape(-1)          # t-major: i = t*8 + b
    idx = np.tile(idx_lin.reshape(NTOK // 16, 16).T, (8, 1)).astype(np.int16)
    return {"emb": embedding, "idx": idx, "wihT": wihT, "whhT": whhT,
            "bgi": bgi, "bhhn": bhhn, "ident": ident}


def prep_in_maps(source, embedding, w_ih, w_hh, b_ih, b_hh):
    import ml_dtypes
    source = np.asarray(source)
    embedding = np.asarray(embedding, dtype=np.float32).astype(
        ml_dtypes.bfloat16)
    w_ih = np.asarray(w_ih, dtype=np.float32)
    w_hh = np.asarray(w_hh, dtype=np.float32)
    b_ih = np.asarray(b_ih, dtype=np.float32)
    b_hh = np.asarray(b_hh, dtype=np.float32)

    # gate reorder (r, z, n) -> (r, n, z)
    perm = np.concatenate([np.arange(0, H), np.arange(2 * H, 3 * H),
                           np.arange(H, 2 * H)])
    w_ih = w_ih[perm]
    w_hh = w_hh[perm]
    bias_gi_rzn = b_ih + b_hh        # h-bias folded for r and z; n keeps b_ih
    bias_gi = np.concatenate([bias_gi_rzn[0:H],           # r
                              b_ih[2 * H:3 * H],          # n
                              bias_gi_rzn[H:2 * H]])      # z
    bhh_n = b_hh[2 * H:3 * H]

    wihT = np.ascontiguousarray(w_ih.T).astype(ml_dtypes.bfloat16)
    whhT = np.ascontiguousarray(w_hh.T).astype(ml_dtypes.bfloat16)
    bgi = np.ascontiguousarray(bias_gi.reshape(NJ, 128).T, dtype=np.float32)
    bhhn = np.ascontiguousarray(
        np.repeat(bhh_n.reshape(NK, 128).T[:, :, None], BL, axis=2)
        .reshape(128, NK * BL)).astype(ml_dtypes.bfloat16)
    ident = np.eye(128, dtype=ml_dtypes.bfloat16)
    return [
        _prep_core_inputs(source[c * BL:(c + 1) * BL], embedding, wihT, whhT,
                          bgi, bhhn, ident)
        for c in range(N_CORES)]


def unpack_results(res):
    """res: list of per-core {'outT': [128, T, 80] bf16} -> [T, B, H] f32."""
    outs = []
    for c in range(N_CORES):
        o = res[c]["outT"].astype(np.float32).reshape(128, T, NK, BL)
        outs.append(o.transpose(1, 3, 2, 0).reshape(T, BL, H))
    return np.concatenate(outs, axis=1)


def kernel(source, embedding, w_ih, w_hh, b_ih, b_hh):
    k = _get_compiled()
    in_maps = prep_in_maps(source, embedding, w_ih, w_hh, b_ih, b_hh)
    dev_args = k.put_inputs(in_maps)
    out = k.run(dev_args)
    return unpack_results(k.results(out))
